# revision 18
# baseline (speedup 1.0000x reference)
"""EventGroupSpecificMMoE kernel for 8 Trainium2 NeuronCores.

Strategy (expert-parallel with host-side token dispatch):
  - Only the ACTIVE group's experts affect the output, so instead of the
    reference's dense 6-bank compute we route: each core is assigned one
    of the 6 group banks (popular groups get 2 cores) and receives only
    the tokens of that group (host gather, padded to a common TOKC).
  - Each core computes, for its TOKC tokens: the 2 shared experts + its
    group's 2 experts (4 MLPs D=512->H=1024->D=512), the gate logits
    (+ per-group cond-embedding bias folded in on host), softmax over
    the 4 experts, and the K=3 weighted combines. Output [K, TOKC, D].
  - Host scatters per-core outputs back into [K, B, T, D].

Device layout: activations keep features on partitions (x as xT[D,tok]),
so W1 (D-major) and W2 (H-major) are used in natural layout and no
on-device transposes are needed. Matmuls run as float32r (full PE rate
for moving dim >= 256).
"""

import os
from contextlib import ExitStack

import numpy as np

from concourse import bacc
import concourse.mybir as mybir
import concourse.tile as tile
from concourse.bass_utils import run_bass_kernel_spmd

B, T, D, H = 8, 512, 512, 1024
S, G, NG, C, K, NEV = 2, 2, 6, 32, 3, 50
E = S + G  # 4 experts per token: [shared0, shared1, group0, group1]
KE = K * E
NCORES = 8
N = B * T

f32 = mybir.dt.float32
f32r = mybir.dt.float32r

DT = D // 128  # 4 d-tiles
HT = H // 128  # 8 h-tiles


def _chunks_for(tokc):
    """Split tokc into chunks in [256, 512] (f32r full rate needs >=256)."""
    n512, rem = divmod(tokc, 512)
    if rem == 0:
        out = [512] * n512
    elif rem == 128:
        assert n512 >= 1
        out = [512] * (n512 - 1) + [384, 256]
    else:  # 256 or 384
        out = [512] * n512 + [rem]
    assert sum(out) == tokc and all(256 <= c <= 512 for c in out)
    return out


def _build_program(tokc, b1nz, b2nz):
    ntt = tokc // 128
    chunks = _chunks_for(tokc)
    nc = bacc.Bacc()

    xt_d = nc.dram_tensor("xt", [DT, 128, tokc], f32, kind="ExternalInput")
    w1_d = nc.dram_tensor("w1", [E, DT, 128, H], f32, kind="ExternalInput")
    w2_d = nc.dram_tensor("w2", [E, HT, 128, D], f32, kind="ExternalInput")
    gw_d = nc.dram_tensor("gw", [DT, 128, KE], f32, kind="ExternalInput")
    condb_d = nc.dram_tensor("condb", [1, KE], f32, kind="ExternalInput")
    if b1nz:
        b1_d = nc.dram_tensor("b1", [E * HT, 128], f32, kind="ExternalInput")
    if b2nz:
        b2_d = nc.dram_tensor("b2", [E, 1, D], f32, kind="ExternalInput")
    out_d = nc.dram_tensor("out", [K, tokc, D], f32, kind="ExternalOutput")

    with ExitStack() as ctx:
        tc = ctx.enter_context(tile.TileContext(nc))
        consts = ctx.enter_context(tc.tile_pool(name="consts", bufs=1))
        w1p = ctx.enter_context(tc.tile_pool(name="w1p", bufs=2))
        w2p = ctx.enter_context(tc.tile_pool(name="w2p", bufs=2))
        htp = ctx.enter_context(tc.tile_pool(name="htp", bufs=2))
        smp = ctx.enter_context(tc.tile_pool(name="smp", bufs=2))
        hps = ctx.enter_context(tc.tile_pool(name="hps", bufs=3, space="PSUM"))
        eops = ctx.enter_context(tc.tile_pool(name="eops", bufs=3, space="PSUM"))
        lps = ctx.enter_context(tc.tile_pool(name="lps", bufs=2, space="PSUM"))

        xt = consts.tile([128, DT, tokc], f32r)
        xt_src = xt_d.rearrange("dt p t -> p dt t").bitcast(f32r)

        def dma_xt(tt):
            sl = slice(tt * 128, (tt + 1) * 128)
            nc.sync.dma_start(out=xt[:, :, sl], in_=xt_src[:, :, sl])

        # DMA issue order tracks first-need time: gating needs xt[tt0]+gw,
        # then W1(e0) needs xt of chunk0 + w1[e0] per h-tile, ...
        dma_xt(0)
        gw = consts.tile([128, DT, KE], f32r)
        nc.sync.dma_start(out=gw, in_=gw_d.rearrange("dt p f -> p dt f").bitcast(f32r))
        condb = consts.tile([1, KE], f32r)
        nc.sync.dma_start(out=condb, in_=condb_d[:, :].bitcast(f32r))
        ntt0 = chunks[0] // 128  # tok-tiles of the first W1 chunk
        for tt in range(1, ntt0):
            dma_xt(tt)
        ones_f = consts.tile([1, 128], f32)
        nc.vector.memset(ones_f, 1.0)
        ones = consts.tile([1, 128], f32r)
        nc.vector.tensor_copy(ones, ones_f)

        # PE warm-up: the HAM clock gate keeps the PE at half clock until
        # ~3us of sustained activity. Run throwaway matmuls on zeros while
        # the first input DMAs are still in flight so real matmuls start at
        # full clock.
        wz_f = consts.tile([128, 256], f32)
        nc.vector.memset(wz_f, 0.0)
        wz_r = consts.tile([128, 256], f32r)
        nc.vector.tensor_copy(wz_r, wz_f)
        warm_ps = eops.tile([128, D], f32, tag="eo", name="warm_ps")
        for _ in range(18):
            nc.tensor.matmul(
                warm_ps[:, :256], lhsT=wz_r[:, :128], rhs=wz_r,
                start=True, stop=True,
            )
        if b1nz:
            b1t = consts.tile([128, E * HT], f32)
            nc.sync.dma_start(out=b1t, in_=b1_d.rearrange("eh p -> p eh"))
        if b2nz:
            b2t = consts.tile([1, E, D], f32r)
            nc.sync.dma_start(
                out=b2t, in_=b2_d.rearrange("e o d -> o e d").bitcast(f32r)
            )
        # softmaxed gate weights, per token-tile: [128, tt, k, e]
        wsc = consts.tile([128, ntt, K, E], f32)
        # combine accumulators, [128, k, tt, D]
        acc = consts.tile([128, K, ntt, D], f32)

        # ---- expert weight streams (issued in first-need order)
        w1ts = [w1p.tile([128, DT, H], f32r, tag="w1t", name=f"w1t{e}")
                for e in range(E)]
        w2ts = [w2p.tile([128, HT, D], f32r, tag="w2t", name=f"w2t{e}")
                for e in range(E)]
        w1_src = [w1_d[e].rearrange("dt p h -> p dt h").bitcast(f32r)
                  for e in range(E)]
        w2_src = [w2_d[e].rearrange("ht p d -> p ht d").bitcast(f32r)
                  for e in range(E)]
        def dma_w1(e):  # piecewise so W1 can start on its first h-tile
            for ht in range(HT):
                sl = slice(ht * 128, (ht + 1) * 128)
                nc.sync.dma_start(out=w1ts[e][:, :, sl], in_=w1_src[e][:, :, sl])

        def dma_w2(e):
            for half in range(2):
                sl = slice(half * (HT // 2), (half + 1) * (HT // 2))
                nc.sync.dma_start(out=w2ts[e][:, sl, :], in_=w2_src[e][:, sl, :])

        dma_w1(0)
        for tt in range(ntt0, ntt):  # rest of xt (later W1 chunks + gating)
            dma_xt(tt)
        dma_w2(0)
        for e in range(1, E):
            dma_w1(e)
            dma_w2(e)

        # ---- gating: logits = x @ gw (+ condb via K=1 ones-row), softmax over e
        for tt in range(ntt):
            lg = lps.tile([128, KE], f32, tag="lg")
            for dt in range(DT):
                nc.tensor.matmul(
                    lg,
                    lhsT=xt[:, dt, tt * 128:(tt + 1) * 128],
                    rhs=gw[:, dt, :],
                    start=(dt == 0),
                    stop=False,
                )
            nc.tensor.matmul(
                lg,
                lhsT=ones,
                rhs=condb,
                start=False,
                stop=True,
            )
            ex = smp.tile([128, KE], f32, tag="ex")
            nc.scalar.activation(ex, lg, mybir.ActivationFunctionType.Exp)
            sm = smp.tile([128, K], f32, tag="sm")
            nc.vector.tensor_reduce(
                sm,
                ex.rearrange("p (k e) -> p k e", e=E),
                axis=mybir.AxisListType.X,
                op=mybir.AluOpType.add,
            )
            rc = smp.tile([128, K], f32, tag="rc")
            nc.vector.reciprocal(rc, sm)
            for k in range(K):
                nc.vector.tensor_scalar_mul(
                    wsc[:, tt, k], ex[:, k * E:(k + 1) * E], rc[:, k:k + 1]
                )

        # ---- experts: h = relu(x @ W1 + b1); eo = h @ W2 (+ b2); acc += w * eo
        # W1 and W2 are fused per 256-token chunk so that on the last expert
        # the finished acc tiles (and their output DMAs) spread over the whole
        # phase instead of bunching at the end.
        for e in range(E):
            w1t, w2t = w1ts[e], w2ts[e]
            c0 = 0
            for cn in chunks:
                hTc = htp.tile([128, HT, cn], f32r, tag="hTc")
                for ht in range(HT):
                    h_ps = hps.tile([128, cn], f32, tag="h_ps")
                    for dt in range(DT):
                        nc.tensor.matmul(
                            h_ps,
                            lhsT=w1t[:, dt, ht * 128:(ht + 1) * 128],
                            rhs=xt[:, dt, c0:c0 + cn],
                            start=(dt == 0),
                            stop=(dt == DT - 1),
                        )
                    bias = b1t[:, e * HT + ht:e * HT + ht + 1] if b1nz else 0.0
                    nc.scalar.activation(
                        hTc[:, ht, :], h_ps,
                        mybir.ActivationFunctionType.Relu, bias=bias,
                    )
                for ltt in range(cn // 128):
                    tt = c0 // 128 + ltt
                    eo = eops.tile([128, D], f32, tag="eo")
                    for ht in range(HT):
                        nc.tensor.matmul(
                            eo,
                            lhsT=hTc[:, ht, ltt * 128:(ltt + 1) * 128],
                            rhs=w2t[:, ht, :],
                            start=(ht == 0),
                            stop=(ht == HT - 1) and not b2nz,
                        )
                    if b2nz:
                        nc.tensor.matmul(
                            eo,
                            lhsT=ones,
                            rhs=b2t[:, e, :],
                            start=False,
                            stop=True,
                        )
                    for k in range(K):
                        sc = wsc[:, tt, k, e:e + 1]
                        if e == 0:
                            nc.vector.tensor_scalar_mul(acc[:, k, tt, :], eo, sc)
                        else:
                            nc.vector.scalar_tensor_tensor(
                                acc[:, k, tt, :],
                                in0=eo,
                                scalar=sc,
                                in1=acc[:, k, tt, :],
                                op0=mybir.AluOpType.mult,
                                op1=mybir.AluOpType.add,
                            )
                        if e == E - 1:
                            nc.sync.dma_start(
                                out=out_d[k, tt * 128:(tt + 1) * 128, :],
                                in_=acc[:, k, tt, :],
                            )
                c0 += cn

    nc.compile()
    return nc


_prog_cache = {}


def _get_program(tokc, b1nz, b2nz):
    key = (tokc, b1nz, b2nz)
    if key not in _prog_cache:
        _prog_cache[key] = _build_program(tokc, b1nz, b2nz)
    return _prog_cache[key]


def _apportion(counts):
    """Assign 8 cores to groups with count>0, each such group >=1 core,
    roughly proportional to counts. Returns list of core counts per group."""
    active = [g for g in range(NG) if counts[g] > 0]
    k = {g: 1 for g in active}
    spare = NCORES - len(active)
    assert spare >= 0, "more nonzero groups than cores"
    for _ in range(spare):
        # give the next core to the group with max per-core load
        g = max(active, key=lambda g: counts[g] / k[g])
        k[g] += 1
    return k


def kernel(**inputs):
    """Full-input entry point. The device run is retried — first in-process,
    then in a fresh subprocess (new PJRT client) — because the axon-tunneled
    NeuronCores occasionally come up wedged from a previous client and fault
    with NRT_EXEC_UNIT_UNRECOVERABLE; a fresh attach recovers them."""
    try:
        return _kernel_impl(inputs)
    except Exception as ex:  # noqa: BLE001
        print(f"kernel: device run failed ({type(ex).__name__}); retrying",
              flush=True)
    import time
    time.sleep(10)
    try:
        return _kernel_impl(inputs)
    except Exception as ex:  # noqa: BLE001
        print(f"kernel: in-process retry failed ({type(ex).__name__}); "
              f"falling back to subprocess", flush=True)
    last = None
    for _ in range(2):
        try:
            return _kernel_subprocess(inputs)
        except Exception as ex:  # noqa: BLE001
            last = ex
            time.sleep(15)
    raise last


def _kernel_subprocess(inputs):
    import subprocess
    import sys
    import tempfile

    with tempfile.TemporaryDirectory() as td:
        in_npz = f"{td}/in.npz"
        out_npy = f"{td}/out.npy"
        np.savez(in_npz, **inputs)
        subprocess.run(
            [sys.executable, os.path.abspath(__file__), "--subproc", in_npz,
             out_npy],
            check=True,
        )
        return np.load(out_npy)


def _kernel_impl(inputs):
    x = np.ascontiguousarray(np.asarray(inputs["x"], dtype=np.float32))
    next_type_ids = np.asarray(inputs["next_type_ids"]).astype(np.int32)
    etype_to_group = np.asarray(inputs["etype_to_group"]).astype(np.int32)
    Ws1 = np.asarray(inputs["Ws1"], dtype=np.float32)
    bs1 = np.asarray(inputs["bs1"], dtype=np.float32)
    Ws2 = np.asarray(inputs["Ws2"], dtype=np.float32)
    bs2 = np.asarray(inputs["bs2"], dtype=np.float32)
    Wg1 = np.asarray(inputs["Wg1"], dtype=np.float32)
    bg1 = np.asarray(inputs["bg1"], dtype=np.float32)
    Wg2 = np.asarray(inputs["Wg2"], dtype=np.float32)
    bg2 = np.asarray(inputs["bg2"], dtype=np.float32)
    cond_emb = np.asarray(inputs["cond_emb"], dtype=np.float32)
    gate_W = np.asarray(inputs["gate_W"], dtype=np.float32)
    gate_b = np.asarray(inputs["gate_b"], dtype=np.float32)

    # ---- routing on host
    safe = np.clip(next_type_ids.reshape(-1), 0, NEV - 1)
    g_tok = etype_to_group[safe]  # [N]
    counts = np.bincount(g_tok, minlength=NG)
    k_per_group = _apportion(counts)

    # per-core (group, token_indices)
    assignments = []  # (gid, idx array)
    for gid, kg in k_per_group.items():
        idx = np.nonzero(g_tok == gid)[0]
        parts = np.array_split(idx, kg)
        for p in parts:
            assignments.append((gid, p))
    while len(assignments) < NCORES:  # only if fewer active groups than cores
        assignments.append((0, np.empty(0, dtype=np.int64)))

    tokc = max(256, -(-max(len(p) for _, p in assignments) // 128) * 128)

    b1nz = bool(np.any(bs1) or np.any(bg1))
    b2nz = bool(np.any(bs2) or np.any(bg2))
    nc = _get_program(tokc, b1nz, b2nz)

    # ---- build per-core inputs
    x_flat = x.reshape(N, D)
    gwx = np.ascontiguousarray(
        gate_W[:, :D, :].transpose(1, 0, 2).reshape(D, KE)
    )  # [D, k*E+e]
    gw_in = np.ascontiguousarray(gwx.reshape(DT, 128, KE))

    in_maps = []
    for gid, idx in assignments:
        xt = np.zeros((D, tokc), dtype=np.float32)
        if len(idx):
            xt[:, :len(idx)] = x_flat[idx].T
        w1 = np.stack([Ws1[0], Ws1[1], Wg1[gid, 0], Wg1[gid, 1]])  # [E, D, H]
        w2 = np.stack([Ws2[0], Ws2[1], Wg2[gid, 0], Wg2[gid, 1]])  # [E, H, D]
        condb = (cond_emb[gid] @ gate_W[:, D:, :] + gate_b).reshape(1, KE)
        m = {
            "xt": np.ascontiguousarray(xt.reshape(DT, 128, tokc)),
            "w1": np.ascontiguousarray(w1.reshape(E, DT, 128, H)),
            "w2": np.ascontiguousarray(w2.reshape(E, HT, 128, D)),
            "gw": gw_in,
            "condb": np.ascontiguousarray(condb.astype(np.float32)),
        }
        if b1nz:
            b1 = np.stack([bs1[0], bs1[1], bg1[gid, 0], bg1[gid, 1]])  # [E, H]
            m["b1"] = np.ascontiguousarray(b1.reshape(E * HT, 128))
        if b2nz:
            b2 = np.stack([bs2[0], bs2[1], bg2[gid, 0], bg2[gid, 1]])  # [E, D]
            m["b2"] = np.ascontiguousarray(b2.reshape(E, 1, D))
        in_maps.append(m)

    res = run_bass_kernel_spmd(nc, in_maps, core_ids=list(range(NCORES))).results

    # ---- unshard
    full = np.empty((K, N, D), dtype=np.float32)
    for (gid, idx), r in zip(assignments, res):
        if len(idx):
            full[:, idx, :] = r["out"][:, :len(idx), :]
    return full.reshape(K, B, T, D)


if __name__ == "__main__":
    import sys

    if len(sys.argv) == 4 and sys.argv[1] == "--subproc":
        _d = np.load(sys.argv[2])
        _out = _kernel_impl({k: _d[k] for k in _d.files})
        np.save(sys.argv[3], _out)
